# revision 16
# baseline (speedup 1.0000x reference)
"""Causal self-attention (B=4, T=2048, C=1024, NH=16) on 8 TRN2 NeuronCores.

Sharding: core = 2*b + g  (b in 0..3 batches, g in 0..1 head-groups of 8 heads).
Each core computes the qkv projection for its 8 heads, causal flash attention,
and a partial output projection (rows g*512:(g+1)*512 of w_proj).  Host sums
the two partials per batch and adds b_proj.

Device layouts (per core):
  x^T    : pre-transposed + pre-tiled on the host to [128, NTB, NCC, 512] bf16
           so every DMA is 128 partitions x 8KB contiguous lines.
  qT, kT : [head-dims on partitions, T on free]  (from  W.T @ x.T  matmuls)
  v      : natural [T on partitions, head-dims on free] with a ones-column
           per head so the PV matmul also produces the softmax denominator
  S^T    : [keys on partitions, queries on free]; both heads of a pair go to
           one 2-bank PSUM tile so a single ScalarE exp covers the pair (no
           max subtraction needed: |S/8| <~ 6 for N(0,1) logits).  Diagonal
           tiles are trimmed to the live query range; masks applied with a
           DVE mask-multiply (hp0) and gpsimd affine_select (hp1) in parallel.

Schedule: the qkv projection for T-block tb+1 is interleaved into the
attention kc-loop of query-block qb=tb, so TensorE always has independent
matmul work while ScalarE churns through the exp stream (which is the
per-kc critical path).  PV runs one kc behind S for the same reason.
"""

import numpy as np

import concourse.bass as bass
import concourse.mybir as mybir
import concourse.tile as tile
from concourse import bacc
from concourse.bass_utils import run_bass_kernel_spmd

B, T, C = 4, 2048, 1024
NH, HD = 16, 64
G = 2              # head groups (cores per batch)
HPG = NH // G      # heads per group = 8
GD = HPG * HD      # dims per group = 512
N_CORES = B * G

FP32 = mybir.dt.float32
ST = mybir.dt.bfloat16

NCC = C // 128      # 8 contraction chunks for the qkv projection
NMB = GD // 128     # 4 blocks of 128 qkv-dims per section
NTB = T // 512      # 4 T-blocks of 512
NKC = T // 128      # 16 key chunks of 128


def build_nc():
    nc = bacc.Bacc()

    xT = nc.declare_dram_parameter("xT", [128, NTB, NCC, 512], ST, isOutput=False)
    wq = nc.declare_dram_parameter("wq", [128, NCC, GD], ST, isOutput=False)
    wk = nc.declare_dram_parameter("wk", [128, NCC, GD], ST, isOutput=False)
    wv = nc.declare_dram_parameter("wv", [128, NCC, GD], ST, isOutput=False)
    bqc = nc.declare_dram_parameter("bqc", [128, NMB], FP32, isOutput=False)
    bkc = nc.declare_dram_parameter("bkc", [128, NMB], FP32, isOutput=False)
    bv = nc.declare_dram_parameter("bv", [GD], FP32, isOutput=False)
    cmask = nc.declare_dram_parameter("cmask", [128, 4, 1024], ST, isOutput=False)
    wp = nc.declare_dram_parameter("wp", [128, NMB, C], ST, isOutput=False)
    out = nc.declare_dram_parameter("out", [T, C], FP32, isOutput=True)

    from contextlib import ExitStack

    with tile.TileContext(nc) as tc, ExitStack() as stack:
        consts = stack.enter_context(tc.tile_pool(name="consts", bufs=1))
        persist = stack.enter_context(tc.tile_pool(name="persist", bufs=1))
        wA_pool = stack.enter_context(tc.tile_pool(name="wA", bufs=1))
        xT_pool = stack.enter_context(tc.tile_pool(name="xTp", bufs=2))
        pT_pool = stack.enter_context(tc.tile_pool(name="pT", bufs=8))
        rec_pool = stack.enter_context(tc.tile_pool(name="rec", bufs=4))
        wp_pool = stack.enter_context(tc.tile_pool(name="wpp", bufs=1))
        osb_pool = stack.enter_context(tc.tile_pool(name="osb", bufs=4))
        sps_pool = stack.enter_context(tc.tile_pool(name="sps", bufs=2, space="PSUM"))
        pvps_pool = stack.enter_context(tc.tile_pool(name="pvps", bufs=2, space="PSUM"))
        gps_pool = stack.enter_context(tc.tile_pool(name="gps", bufs=2, space="PSUM"))

        bq_col = consts.tile([128, NMB], FP32, tag="bq_col")
        bk_col = consts.tile([128, NMB], FP32, tag="bk_col")
        nc.sync.dma_start(out=bq_col, in_=bqc[:, :])
        nc.sync.dma_start(out=bk_col, in_=bkc[:, :])
        # causal keep-masks for the two heads of a diagonal pair tile, one
        # per diagonal offset r (host-precomputed)
        mask_t = consts.tile([128, 4, 1024], ST, tag="mask_t")
        # preload the exp activation-table set during stage A
        scrap = consts.tile([1, 16], FP32, tag="scrap")
        nc.vector.memset(scrap, 0.0)
        nc.scalar.activation(
            out=scrap, in_=scrap, func=mybir.ActivationFunctionType.Exp
        )

        # persistent activations
        qT_t = [persist.tile([128, T], ST, tag=f"qT{m}", name=f"qT{m}") for m in range(NMB)]
        kT_t = [persist.tile([128, T], ST, tag=f"kT{m}", name=f"kT{m}") for m in range(NMB)]
        v_all = persist.tile([128, NKC, HPG, HD + 1], ST, tag="v_all", name="v_all")
        # softmax-denominator ones column of every head
        nc.vector.memset(v_all[:, :, :, HD : HD + 1], 1.0)
        # y^T reuses the qT tiles: the query columns of head-pair m, block qb
        # are dead once that block's PV matmuls have consumed them.
        yT_t = qT_t

        # ---- stage A inputs ----
        # Per-chunk DMAs so the first matmuls start after ~one chunk, not
        # after whole-tensor transfers.  sync queue: x blocks + wk; scalar
        # queue: wq, bv, wv, wp — balanced so each consumer's data lands
        # just ahead of its first use.
        xtc = [None] * NTB
        wq_t = wA_pool.tile([128, NCC, GD], ST, tag="wq")
        wk_t = wA_pool.tile([128, NCC, GD], ST, tag="wk")
        wv_t = wA_pool.tile([128, NCC, GD], ST, tag="wv")
        wp_t = wp_pool.tile([128, NMB, C], ST, tag="wp")
        bv_bc = consts.tile([128, GD], FP32, tag="bv_bc")

        def dma_xtc(tb):
            xtc[tb] = xT_pool.tile([128, NCC, 512], ST, tag="xtc", name="xtc")
            for c in range(NCC):
                nc.sync.dma_start(out=xtc[tb][:, c, :], in_=xT[:, tb, c, :])

        dma_xtc(0)
        for c in range(NCC):
            nc.scalar.dma_start(out=wq_t[:, c, :], in_=wq[:, c, :])
        for c in range(NCC):
            nc.sync.dma_start(out=wk_t[:, c, :], in_=wk[:, c, :])
        # bv broadcast to all 128 partitions (DMA supports partition step 0)
        nc.scalar.dma_start(out=bv_bc, in_=bv[None, :].partition_broadcast(128))
        for c in range(NCC):
            nc.scalar.dma_start(out=wv_t[:, c, :], in_=wv[:, c, :])
        dma_xtc(1)
        nc.scalar.dma_start(out=wp_t, in_=wp[:, :, :])
        nc.scalar.dma_start(out=mask_t, in_=cmask[:, :, :])

        def emit_qk_group(tb, w_t, b_col, dst, m):
            ps = gps_pool.tile([128, 512], FP32, tag="gps", name="gps")
            for c in range(NCC):
                nc.tensor.matmul(
                    ps,
                    w_t[:, c, bass.ts(m, 128)],
                    xtc[tb][:, c, :],
                    start=(c == 0),
                    stop=(c == NCC - 1),
                )
            nc.vector.tensor_scalar_add(
                dst[m][:, bass.ts(tb, 512)], ps, b_col[:, m : m + 1]
            )

        def emit_v_group(tb, tsub):
            kc = tb * 4 + tsub
            ps = gps_pool.tile([128, GD], FP32, tag="gps", name="gps")
            for c in range(NCC):
                nc.tensor.matmul(
                    ps,
                    xtc[tb][:, c, bass.ts(tsub, 128)],
                    wv_t[:, c, :],
                    start=(c == 0),
                    stop=(c == NCC - 1),
                )
            vt = v_all[:, kc, :, :]
            nc.vector.tensor_add(
                vt[:, :, 0:HD],
                ps.rearrange("p (h d) -> p h d", h=HPG),
                bv_bc.rearrange("p (h d) -> p h d", h=HPG),
            )

        def stage_a_groups(tb):
            groups = []
            for w_t, b_col, dst in ((wq_t, bq_col, qT_t), (wk_t, bk_col, kT_t)):
                for m in range(NMB):
                    groups.append(lambda tb=tb, w=w_t, b=b_col, d=dst, m=m: emit_qk_group(tb, w, b, d, m))
            for tsub in range(4):
                groups.append(lambda tb=tb, t=tsub: emit_v_group(tb, t))
            return groups

        # ---------------- stage A for tb=0 (solo intro) ----------------
        for g in stage_a_groups(0):
            g()

        def emit_proj_group(tb16, nb):
            ps = gps_pool.tile([128, 512], FP32, tag="gps", name="gps")
            for c in range(NMB):
                nc.tensor.matmul(
                    ps,
                    yT_t[c][:, bass.ts(tb16, 128)],
                    wp_t[:, c, bass.ts(nb, 512)],
                    start=(c == 0),
                    stop=(c == NMB - 1),
                )
            osb = osb_pool.tile([128, 512], FP32, tag="osb", name="osb")
            if tb16 >= 12 and (tb16 * 2 + nb) % 2 == 1:
                nc.scalar.copy(osb, ps)
            else:
                nc.vector.tensor_copy(osb, ps)
            eng = nc.sync if (tb16 * 2 + nb) % 2 == 0 else nc.gpsimd
            eng.dma_start(out=out[bass.ts(tb16, 128), bass.ts(nb, 512)], in_=osb)

        deferred_proj = []

        # ---------------- attention, with A(tb=qb+1) interleaved ----------------
        for qb in range(NTB):
            kcmax = (qb + 1) * 4
            a_work = []
            if qb + 2 < NTB:
                dma_xtc(qb + 2)
            if qb + 1 < NTB:
                a_work = stage_a_groups(qb + 1)
            else:
                a_work = deferred_proj
            n_kc_total = NMB * kcmax
            a_credit = 1.0
            a_per_kc = len(a_work) / n_kc_total if n_kc_total else 0.0

            for m in range(NMB):
                pvs = [
                    pvps_pool.tile([HD + 1, 512], FP32, tag="pvps", name="pvps")
                    for _ in range(2)
                ]
                prev = None
                for kc in range(kcmax):
                    r = kc - qb * 4
                    off = 128 * r if r > 0 else 0
                    n = 512 - off
                    sp2 = sps_pool.tile([128, 1024], FP32, tag="sps", name="sp2")
                    for hp in range(2):
                        base = hp * 64
                        nc.tensor.matmul(
                            sp2[:, 512 * hp + off : 512 * hp + 512],
                            kT_t[m][base : base + 64, bass.ts(kc, 128)],
                            qT_t[m][
                                base : base + 64,
                                512 * qb + off : 512 * qb + 512,
                            ],
                            start=True,
                            stop=True,
                        )
                    pT2 = pT_pool.tile([128, 1024], ST, tag="pT", name="pT2")
                    nc.scalar.activation(
                        out=pT2[:, off:1024],
                        in_=sp2[:, off:1024],
                        func=mybir.ActivationFunctionType.Exp,
                        scale=1.0 / float(np.sqrt(HD)),
                    )
                    if r >= 0:
                        # keep key j <= local query f, both heads in one op
                        nc.vector.tensor_mul(
                            pT2[:, off:1024],
                            pT2[:, off:1024],
                            mask_t[:, r, off:1024],
                        )
                    if prev is not None:
                        pkc, ppT2, poff = prev
                        for hp in range(2):
                            nc.tensor.matmul(
                                pvs[hp][:, poff:512],
                                v_all[:, pkc, 2 * m + hp, :],
                                ppT2[:, 512 * hp + poff : 512 * hp + 512],
                                start=(pkc == 0),
                                stop=False,
                                skip_group_check=True,
                            )
                    # interleave stage-A matmul groups to keep TensorE fed
                    a_credit += a_per_kc
                    while a_credit >= 1.0 and a_work:
                        a_work.pop(0)()
                        a_credit -= 1.0
                    prev = (kc, pT2, off)
                pkc, ppT2, poff = prev
                for hp in range(2):
                    nc.tensor.matmul(
                        pvs[hp][:, poff:512],
                        v_all[:, pkc, 2 * m + hp, :],
                        ppT2[:, 512 * hp + poff : 512 * hp + 512],
                        start=(pkc == 0),
                        stop=True,
                        skip_group_check=True,
                    )
                last = qb == NTB - 1 and m == NMB - 1
                if last:
                    # keep the PE's HAM activity window busy through the
                    # final normalize chain so the closing projection
                    # matmuls run at the warm clock
                    for dk in range(8):
                        dsp = sps_pool.tile([128, 1024], FP32, tag="sps", name="dsp")
                        nc.tensor.matmul(
                            dsp[:, 0:512],
                            kT_t[0][0:64, 0:128],
                            qT_t[0][0:64, 0:512],
                            start=True,
                            stop=True,
                        )
                for hp in range(2):
                    base = hp * 64
                    # 1/denominator (row 64 of the PV accumulator), then
                    # unnormalized y^T out of PSUM; normalize in SBUF.
                    den_s = rec_pool.tile([1, 512], FP32, tag="den_s", name="den_s")
                    if last and hp == 0:
                        nc.scalar.copy(den_s, pvs[hp][HD : HD + 1, :])
                    else:
                        nc.vector.tensor_copy(den_s, pvs[hp][HD : HD + 1, :])
                    r_row = rec_pool.tile([1, 512], FP32, tag="r_row", name="r_row")
                    nc.vector.reciprocal_approx_fast(r_row, den_s)
                    rbc = rec_pool.tile([128, 512], FP32, tag="rbc", name="rbc")
                    nc.gpsimd.partition_broadcast(rbc, r_row)
                    if last:
                        # tail: fuse the PSUM copy and normalize multiply
                        nc.vector.tensor_mul(
                            yT_t[m][base : base + 64, bass.ts(qb, 512)],
                            pvs[hp][0:HD, :],
                            rbc[0:HD, :],
                        )
                    else:
                        nc.vector.tensor_copy(
                            yT_t[m][base : base + 64, bass.ts(qb, 512)],
                            pvs[hp][0:HD, :],
                        )
                        nc.vector.tensor_mul(
                            yT_t[m][base : base + 64, bass.ts(qb, 512)],
                            yT_t[m][base : base + 64, bass.ts(qb, 512)],
                            rbc[base : base + 64, :],
                        )
            # flush any remaining interleaved stage-A groups
            for g in a_work:
                g()
            # output projection: qb<3 groups are deferred into qb=3's kc
            # loop as TensorE filler (that phase is exp-bound)
            for tsub in range(4):
                tb16 = qb * 4 + tsub
                for nb in range(C // 512):
                    if qb < NTB - 1:
                        deferred_proj.append(
                            lambda t=tb16, n=nb: emit_proj_group(t, n)
                        )
                    else:
                        emit_proj_group(tb16, nb)

    nc.compile()
    return nc


_CACHE = {}


def _get_nc():
    if "nc" not in _CACHE:
        _CACHE["nc"] = build_nc()
    return _CACHE["nc"]


def _to_st(a):
    a = np.asarray(a, dtype=np.float32)
    import ml_dtypes

    return np.ascontiguousarray(a.astype(ml_dtypes.bfloat16))


def _wtile(w):
    # [C', N] -> [128, C'//128, N] partition-major contiguous
    cc, n = w.shape
    return np.ascontiguousarray(w.reshape(cc // 128, 128, n).transpose(1, 0, 2))


def make_in_maps(x, w_qkv, b_qkv, w_proj):
    x = np.asarray(x, dtype=np.float32)
    w_qkv = np.asarray(w_qkv, dtype=np.float32)
    b_qkv = np.asarray(b_qkv, dtype=np.float32)
    w_proj = np.asarray(w_proj, dtype=np.float32)
    # causal keep-masks per diagonal offset r: cols [off:512) keep jl <= j-off
    # for head 0, cols [512+off:1024) keep jl <= j-512-off for head 1; the
    # stale-garbage zone [512:512+off) is zeroed.
    import ml_dtypes
    cm = np.zeros((128, 4, 1024), dtype=np.float32)
    jl = np.arange(128)[:, None]
    for r in range(4):
        off = 128 * r
        j0 = np.arange(off, 512)[None, :]
        cm[:, r, off:512] = (jl <= (j0 - off)).astype(np.float32)
        j1 = np.arange(512 + off, 1024)[None, :]
        cm[:, r, 512 + off : 1024] = (jl <= (j1 - 512 - off)).astype(np.float32)
    cm = np.ascontiguousarray(cm.astype(ml_dtypes.bfloat16))
    xTs = []
    for b in range(B):
        xt = x[b].T  # [C, T]
        # [128, NTB, NCC, 512]: xt[c*128+p, tb*512+t]
        xTs.append(
            np.ascontiguousarray(
                xt.reshape(NCC, 128, NTB, 512).transpose(1, 2, 0, 3).astype(
                    __import__("ml_dtypes").bfloat16
                )
            )
        )
    in_maps = []
    for core in range(N_CORES):
        b, g = divmod(core, G)
        bq = b_qkv[GD * g : GD * g + GD]
        bk = b_qkv[C + GD * g : C + GD * g + GD]
        in_maps.append(
            {
                "xT": xTs[b],
                "wq": _to_st(_wtile(w_qkv[:, GD * g : GD * g + GD])),
                "wk": _to_st(_wtile(w_qkv[:, C + GD * g : C + GD * g + GD])),
                "wv": _to_st(_wtile(w_qkv[:, 2 * C + GD * g : 2 * C + GD * g + GD])),
                "bqc": np.ascontiguousarray(bq.reshape(NMB, 128).T.astype(np.float32)),
                "bkc": np.ascontiguousarray(bk.reshape(NMB, 128).T.astype(np.float32)),
                "bv": np.ascontiguousarray(b_qkv[2 * C + GD * g : 2 * C + GD * g + GD]),
                "wp": _to_st(_wtile(w_proj[GD * g : GD * g + GD, :])),
                "cmask": cm,
            }
        )
    return in_maps


def _assemble(results, b_proj):
    y = np.empty((B, T, C), dtype=np.float32)
    for b in range(B):
        y[b] = results[G * b]["out"] + results[G * b + 1]["out"]
    y += np.asarray(b_proj, dtype=np.float32)[None, None, :]
    return y


def kernel(x, w_qkv, b_qkv, w_proj, b_proj):
    nc = _get_nc()
    in_maps = make_in_maps(x, w_qkv, b_qkv, w_proj)
    res = run_bass_kernel_spmd(nc, in_maps, list(range(N_CORES)))
    return _assemble(res.results, b_proj)
